# revision 1
# baseline (speedup 1.0000x reference)
"""LIF layer (T=64, B=128, 2048->2048) on 8 trn2 NeuronCores.

Strategy: tensor-parallel over out_dim (each core owns 256 output
channels, sees the full x_seq). Per core:
  GEMM  cur[o, t*B+b] = sum_i W[o,i] * x[t,b,i]   (W stationary in PE)
  SCAN  64 sequential LIF steps on [128, 2, 128] tiles (DVE), reading
        cur straight out of PSUM.
Bias is folded away via the change of variable u = mem - b/(1-decay),
turning the per-step bias add into a per-channel spike threshold.

Host-side prep (not on HW): transpose x to [I, T*B], slice/transpose W,
precompute threshold/init tiles, final output is a cheap transpose+concat.
"""

import math
import os

import numpy as np

import concourse.bacc as bacc
import concourse.bass as bass
import concourse.mybir as mybir
import concourse.tile as tile
from concourse import bass_utils

# Problem constants (hardcoded per contract)
T, B, I, O = 64, 128, 2048, 2048
N_CORES = 8
OL = O // N_CORES          # 256 out-channels per core
TB = T * B                 # 8192 rows
KT = I // 128              # 16 k-tiles
NPB_COLS = 1024            # tb-columns per block (= 8 timesteps)
N_NPB = TB // NPB_COLS     # 8 blocks
MM_N = 512                 # moving free dim per matmul (psum bank)
TAU, THR = 2.0, 1.0
DECAY = math.exp(-1.0 / TAU)

F32 = mybir.dt.float32
ALU = mybir.AluOpType

# GEMM precision mode: "fp32" (exact, 4 cyc/row), "f32r" (fp22 inputs,
# 1 cyc/row), "bf16x3" (3-pass hi/lo split, ~exact, 3 cyc/row)
MODE = os.environ.get("LIF_MODE", "bf16x3")

_cache = {}


def _build_nc(mode):
    nc = bacc.Bacc(trn_type="TRN2", target_bir_lowering=False)

    mm_dt = {"fp32": F32, "f32r": mybir.dt.float32r, "bf16x3": mybir.dt.bfloat16}[mode]

    # DRAM I/O. "stages": list of (x_dram, [w_variant_idx,...]) — each stage
    # loads its x tile once and runs matmuls against each listed w variant,
    # all accumulating into the same PSUM group.
    if mode == "bf16x3":
        xT_h = nc.dram_tensor("xT_h", [I, TB], mybir.dt.bfloat16, kind="ExternalInput")
        xT_l = nc.dram_tensor("xT_l", [I, TB], mybir.dt.bfloat16, kind="ExternalInput")
        n_wv = 2
        stages = [(xT_h, [0, 1]), (xT_l, [0])]   # xh@wh + xh@wl + xl@wh
    else:
        xT = nc.dram_tensor("xT", [I, TB], F32, kind="ExternalInput")
        n_wv = 1
        stages = [(xT, [0])]
    # weights pre-packed on host to w_all's exact SBUF layout -> one fast DMA
    w_packed = nc.dram_tensor("w_packed", [128, n_wv, KT, 2, 128], mm_dt,
                              kind="ExternalInput")
    n_mm_grp = sum(len(wvs) for _, wvs in stages) * KT  # accum group length
    thr_d = nc.dram_tensor("thr", [128, 2, 128], F32, kind="ExternalInput")
    u0_d = nc.dram_tensor("u0", [128, 2, 128], F32, kind="ExternalInput")
    out_d = nc.dram_tensor("out", [128, 2, T, B], F32, kind="ExternalOutput")

    with tile.TileContext(nc) as tc:
        with (
            tc.tile_pool(name="wpool", bufs=1) as wpool,
            tc.tile_pool(name="xpool", bufs=8) as xpool,
            tc.tile_pool(name="state", bufs=1) as state,
            tc.tile_pool(name="spkpool", bufs=4) as spkpool,
            tc.tile_pool(name="psum", bufs=8, space="PSUM") as psum_pool,
        ):
            # Preload weight tiles, one contiguous DMA per variant (gpsimd
            # queue, parallel to the x-prefetch on the sync queue); first
            # matmul only gates on variant 0.
            w_all = wpool.tile([128, n_wv, KT, 2, 128], mm_dt)
            for wv in range(n_wv):
                nc.gpsimd.dma_start(w_all[:, wv], w_packed[:, wv])

            # Persistent state tiles
            u = state.tile([128, 2, 128], F32)
            thr_t = state.tile([128, 2, 128], F32)
            nc.gpsimd.dma_start(u[:], u0_d[:])
            nc.gpsimd.dma_start(thr_t[:], thr_d[:])

            # col-blocks: 1024-wide except the last 1024 split in two, so the
            # final exposed scan (after the last matmul) is only 4 steps
            blocks = [(i * NPB_COLS, NPB_COLS) for i in range(N_NPB - 1)]
            blocks += [(TB - 1024, 512), (TB - 512, 512)]
            for bi, (cs, ncols) in enumerate(blocks):
                n_nn = ncols // MM_N
                # one psum tile per (ot, nn): [128, 512] fp32 = 1 bank
                ps = [[psum_pool.tile([128, MM_N], F32, tag="ps", name=f"ps_{bi}_{ot}_{nn}")
                       for nn in range(n_nn)] for ot in range(2)]
                mm_i = 0
                for x_src, wvs in stages:
                    for k in range(KT):
                        xt = xpool.tile([128, NPB_COLS], mm_dt, tag="xt",
                                        name=f"xt_{bi}_{k}")
                        nc.sync.dma_start(
                            xt[:, :ncols],
                            x_src[k * 128:(k + 1) * 128, cs:cs + ncols],
                        )
                        for wv in wvs:
                            mm_i += 1
                            for ot in range(2):
                                for nn in range(n_nn):
                                    nc.tensor.matmul(
                                        ps[ot][nn][:],
                                        w_all[:, wv, k, ot, :],
                                        xt[:, nn * MM_N:(nn + 1) * MM_N],
                                        start=(mm_i == 1),
                                        stop=(mm_i == n_mm_grp),
                                    )

                # LIF steps consuming this block's PSUM
                for tl in range(ncols // 128):
                    t = (cs // 128) + tl  # global timestep
                    nn, off = tl // 4, (tl % 4) * 128
                    nc.vector.tensor_scalar_mul(u[:], u[:], DECAY)
                    nc.vector.tensor_tensor(
                        u[:, 0, :], u[:, 0, :], ps[0][nn][:, off:off + 128], op=ALU.add)
                    nc.vector.tensor_tensor(
                        u[:, 1, :], u[:, 1, :], ps[1][nn][:, off:off + 128], op=ALU.add)
                    spk = spkpool.tile([128, 2, 128], F32, tag="spk")
                    nc.vector.tensor_tensor(spk[:], u[:], thr_t[:], op=ALU.is_gt)
                    nc.vector.tensor_tensor(u[:], u[:], spk[:], op=ALU.subtract)
                    nc.gpsimd.dma_start(out_d[:, :, t, :], spk[:])

    nc.compile()
    return nc


def _get_nc(mode):
    if mode not in _cache:
        _cache[mode] = _build_nc(mode)
    return _cache[mode]


def kernel(x_seq: np.ndarray, W: np.ndarray, b: np.ndarray) -> np.ndarray:
    mode = MODE
    nc = _get_nc(mode)

    x = np.ascontiguousarray(x_seq.reshape(TB, I), dtype=np.float32)
    xT = np.ascontiguousarray(x.T)  # [I, TB]

    if mode == "bf16x3":
        import ml_dtypes
        xT_h = xT.astype(ml_dtypes.bfloat16)
        xT_l = (xT - xT_h.astype(np.float32)).astype(ml_dtypes.bfloat16)

    in_maps = []
    for c in range(N_CORES):
        w_c = W[c * OL:(c + 1) * OL, :].astype(np.float32)      # [OL, I]
        wTc = np.ascontiguousarray(w_c.T)                       # [I, OL]
        b_c = b[c * OL:(c + 1) * OL].astype(np.float32)         # [OL]
        shift = b_c / (1.0 - DECAY)
        thr = (THR - shift).reshape(2, 128).transpose(1, 0)     # [128(op), 2(ot)]
        u0 = (-shift).reshape(2, 128).transpose(1, 0)
        thr_tile = np.ascontiguousarray(
            np.broadcast_to(thr[:, :, None], (128, 2, 128)), dtype=np.float32)
        u0_tile = np.ascontiguousarray(
            np.broadcast_to(u0[:, :, None], (128, 2, 128)), dtype=np.float32)
        m = {"thr": thr_tile, "u0": u0_tile}

        def pack_w(wt):  # [I, OL] -> [128(p), KT, 2(ot), 128(f)]
            return wt.reshape(KT, 128, 2, 128).transpose(1, 0, 2, 3)

        if mode == "bf16x3":
            wTc_h = wTc.astype(ml_dtypes.bfloat16)
            wTc_l = (wTc - wTc_h.astype(np.float32)).astype(ml_dtypes.bfloat16)
            wp = np.ascontiguousarray(
                np.stack([pack_w(wTc_h), pack_w(wTc_l)], axis=1))
            m.update(xT_h=xT_h, xT_l=xT_l, w_packed=wp)
        else:
            wp = np.ascontiguousarray(pack_w(wTc)[:, None])
            m.update(xT=xT, w_packed=wp)
        in_maps.append(m)

    res = bass_utils.run_bass_kernel_spmd(nc, in_maps, core_ids=list(range(N_CORES)))
    global LAST_RESULT
    LAST_RESULT = res

    # Assemble: out_c[op, ot, t, b] -> [t, b, ot*128+op]; concat over cores
    parts = []
    for c in range(N_CORES):
        oc = res.results[c]["out"]  # [128, 2, T, B]
        parts.append(oc.transpose(2, 3, 1, 0).reshape(T, B, 2 * 128))
    return np.ascontiguousarray(np.concatenate(parts, axis=2))


LAST_RESULT = None



# revision 2
# speedup vs baseline: 1.6911x; 1.6911x over previous
"""LIF layer (T=64, B=128, 2048->2048) on 8 trn2 NeuronCores.

Strategy: tensor-parallel over out_dim (each core owns 256 output
channels, sees the full x_seq). Per core:
  GEMM  cur[o, t*B+b] = sum_i W[o,i] * x[t,b,i]   (W stationary in PE)
  SCAN  64 sequential LIF steps on [128, 2, 128] tiles (DVE), reading
        cur straight out of PSUM.
Bias is folded away via the change of variable u = mem - b/(1-decay),
turning the per-step bias add into a per-channel spike threshold.

Host-side prep (not on HW): transpose x to [I, T*B], slice/transpose W,
precompute threshold/init tiles, final output is a cheap transpose+concat.
"""

import math
import os

import numpy as np

import concourse.bacc as bacc
import concourse.bass as bass
import concourse.mybir as mybir
import concourse.tile as tile
from concourse import bass_utils

# Problem constants (hardcoded per contract)
T, B, I, O = 64, 128, 2048, 2048
N_CORES = 8
OL = O // N_CORES          # 256 out-channels per core
TB = T * B                 # 8192 rows
KT = I // 128              # 16 k-tiles
NPB_COLS = 1024            # tb-columns per block (= 8 timesteps)
N_NPB = TB // NPB_COLS     # 8 blocks
MM_N = 512                 # moving free dim per matmul (psum bank)
TAU, THR = 2.0, 1.0
DECAY = math.exp(-1.0 / TAU)

F32 = mybir.dt.float32
ALU = mybir.AluOpType

# GEMM precision mode: "fp32" (exact, 4 cyc/row), "f32r" (fp22 inputs,
# 1 cyc/row), "bf16x3" (3-pass hi/lo split, ~exact, 3 cyc/row)
MODE = os.environ.get("LIF_MODE", "bf16x3")

_cache = {}


def _build_nc(mode):
    nc = bacc.Bacc(trn_type="TRN2", target_bir_lowering=False)

    mm_dt = {"fp32": F32, "f32r": mybir.dt.float32r, "bf16x3": mybir.dt.bfloat16}[mode]

    # DRAM I/O. "stages": list of (x_dram, [w_variant_idx,...]) — each stage
    # loads its x tile once and runs matmuls against each listed w variant,
    # all accumulating into the same PSUM group.
    if mode == "bf16x3":
        xT_h = nc.dram_tensor("xT_h", [I, TB], mybir.dt.bfloat16, kind="ExternalInput")
        xT_l = nc.dram_tensor("xT_l", [I, TB], mybir.dt.bfloat16, kind="ExternalInput")
        n_wv = 2
        stages = [(xT_h, [0, 1]), (xT_l, [0])]   # xh@wh + xh@wl + xl@wh
    else:
        xT = nc.dram_tensor("xT", [I, TB], mm_dt, kind="ExternalInput")
        n_wv = 1
        stages = [(xT, [0])]
    # weights pre-packed on host to w_all's exact SBUF layout -> one fast DMA
    w_packed = nc.dram_tensor("w_packed", [128, n_wv, KT, 2, 128], mm_dt,
                              kind="ExternalInput")
    n_mm_grp = sum(len(wvs) for _, wvs in stages) * KT  # accum group length
    thr_d = nc.dram_tensor("thr", [128, 2, 128], F32, kind="ExternalInput")
    u0_d = nc.dram_tensor("u0", [128, 2, 128], F32, kind="ExternalInput")
    out_d = nc.dram_tensor("out", [128, 2, T, B], F32, kind="ExternalOutput")

    with tile.TileContext(nc) as tc:
        with (
            tc.tile_pool(name="wpool", bufs=1) as wpool,
            tc.tile_pool(name="xpool", bufs=8) as xpool,
            tc.tile_pool(name="state", bufs=1) as state,
            tc.tile_pool(name="spkpool", bufs=4) as spkpool,
            tc.tile_pool(name="psum", bufs=8, space="PSUM") as psum_pool,
        ):
            # Preload weight tiles, one contiguous DMA per variant (gpsimd
            # queue, parallel to the x-prefetch on the sync queue); first
            # matmul only gates on variant 0.
            w_all = wpool.tile([128, n_wv, KT, 2, 128], mm_dt)
            for wv in range(n_wv):
                nc.gpsimd.dma_start(w_all[:, wv], w_packed[:, wv])

            # Persistent state tiles
            u = state.tile([128, 2, 128], F32)
            thr_t = state.tile([128, 2, 128], F32)
            nc.gpsimd.dma_start(u[:], u0_d[:])
            nc.gpsimd.dma_start(thr_t[:], thr_d[:])

            # col-blocks: 1024-wide except the last 1024 split in two, so the
            # final exposed scan (after the last matmul) is only 4 steps
            blocks = [(i * NPB_COLS, NPB_COLS) for i in range(N_NPB - 1)]
            blocks += [(TB - 1024, 512), (TB - 512, 512)]
            for bi, (cs, ncols) in enumerate(blocks):
                n_nn = ncols // MM_N
                # one psum tile per (ot, nn): [128, 512] fp32 = 1 bank
                ps = [[psum_pool.tile([128, MM_N], F32, tag="ps", name=f"ps_{bi}_{ot}_{nn}")
                       for nn in range(n_nn)] for ot in range(2)]
                mm_i = 0
                for x_src, wvs in stages:
                    for k in range(KT):
                        xt = xpool.tile([128, NPB_COLS], mm_dt, tag="xt",
                                        name=f"xt_{bi}_{k}")
                        nc.sync.dma_start(
                            xt[:, :ncols],
                            x_src[k * 128:(k + 1) * 128, cs:cs + ncols],
                        )
                        for wv in wvs:
                            mm_i += 1
                            for ot in range(2):
                                for nn in range(n_nn):
                                    nc.tensor.matmul(
                                        ps[ot][nn][:],
                                        w_all[:, wv, k, ot, :],
                                        xt[:, nn * MM_N:(nn + 1) * MM_N],
                                        start=(mm_i == 1),
                                        stop=(mm_i == n_mm_grp),
                                    )

                # LIF steps consuming this block's PSUM
                for tl in range(ncols // 128):
                    t = (cs // 128) + tl  # global timestep
                    nn, off = tl // 4, (tl % 4) * 128
                    nc.vector.tensor_scalar_mul(u[:], u[:], DECAY)
                    nc.vector.tensor_tensor(
                        u[:, 0, :], u[:, 0, :], ps[0][nn][:, off:off + 128], op=ALU.add)
                    nc.vector.tensor_tensor(
                        u[:, 1, :], u[:, 1, :], ps[1][nn][:, off:off + 128], op=ALU.add)
                    spk = spkpool.tile([128, 2, 128], F32, tag="spk")
                    nc.vector.tensor_tensor(spk[:], u[:], thr_t[:], op=ALU.is_gt)
                    nc.vector.tensor_tensor(u[:], u[:], spk[:], op=ALU.subtract)
                    nc.gpsimd.dma_start(out_d[:, :, t, :], spk[:])

    nc.compile()
    return nc


def _get_nc(mode):
    if mode not in _cache:
        _cache[mode] = _build_nc(mode)
    return _cache[mode]


def kernel(x_seq: np.ndarray, W: np.ndarray, b: np.ndarray) -> np.ndarray:
    mode = MODE
    nc = _get_nc(mode)

    x = np.ascontiguousarray(x_seq.reshape(TB, I), dtype=np.float32)
    xT = np.ascontiguousarray(x.T)  # [I, TB]

    if mode == "bf16x3":
        import ml_dtypes
        xT_h = xT.astype(ml_dtypes.bfloat16)
        xT_l = (xT - xT_h.astype(np.float32)).astype(ml_dtypes.bfloat16)

    in_maps = []
    for c in range(N_CORES):
        w_c = W[c * OL:(c + 1) * OL, :].astype(np.float32)      # [OL, I]
        wTc = np.ascontiguousarray(w_c.T)                       # [I, OL]
        b_c = b[c * OL:(c + 1) * OL].astype(np.float32)         # [OL]
        shift = b_c / (1.0 - DECAY)
        thr = (THR - shift).reshape(2, 128).transpose(1, 0)     # [128(op), 2(ot)]
        u0 = (-shift).reshape(2, 128).transpose(1, 0)
        thr_tile = np.ascontiguousarray(
            np.broadcast_to(thr[:, :, None], (128, 2, 128)), dtype=np.float32)
        u0_tile = np.ascontiguousarray(
            np.broadcast_to(u0[:, :, None], (128, 2, 128)), dtype=np.float32)
        m = {"thr": thr_tile, "u0": u0_tile}

        def pack_w(wt):  # [I, OL] -> [128(p), KT, 2(ot), 128(f)]
            return wt.reshape(KT, 128, 2, 128).transpose(1, 0, 2, 3)

        if mode == "bf16x3":
            wTc_h = wTc.astype(ml_dtypes.bfloat16)
            wTc_l = (wTc - wTc_h.astype(np.float32)).astype(ml_dtypes.bfloat16)
            wp = np.ascontiguousarray(
                np.stack([pack_w(wTc_h), pack_w(wTc_l)], axis=1))
            m.update(xT_h=xT_h, xT_l=xT_l, w_packed=wp)
        else:
            wp = np.ascontiguousarray(pack_w(wTc)[:, None])
            m.update(xT=xT, w_packed=wp)
        in_maps.append(m)

    res = bass_utils.run_bass_kernel_spmd(nc, in_maps, core_ids=list(range(N_CORES)))
    global LAST_RESULT
    LAST_RESULT = res

    # Assemble: out_c[op, ot, t, b] -> [t, b, ot*128+op]; concat over cores
    parts = []
    for c in range(N_CORES):
        oc = res.results[c]["out"]  # [128, 2, T, B]
        parts.append(oc.transpose(2, 3, 1, 0).reshape(T, B, 2 * 128))
    return np.ascontiguousarray(np.concatenate(parts, axis=2))


LAST_RESULT = None



# revision 4
# speedup vs baseline: 1.9760x; 1.1685x over previous
"""LIF layer (T=64, B=128, 2048->2048) on 8 trn2 NeuronCores.

Strategy: hybrid shard = 4-way batch x 2-way out_dim. Core c handles
b-group (c % 4, 32 batches) and o-group (c // 4, 1024 out channels).
Per core:
  GEMM  single-pass float32r (fp22-rounded inputs, 1 cyc/row on PE):
        cur[o, t*32+b] = sum_i W[o,i] * x[t,b,i], W stationary.
        8 col-blocks of 256 (= 8 timesteps), 16 k-tiles x 8 o-tiles,
        PSUM block = 4 banks, double buffered.
  SCAN  64 sequential LIF steps on DVE, 3 fused ops per step:
        u = u*decay + cur (scalar_tensor_tensor, cur strided from PSUM)
        spk = u > thr     (tensor_tensor is_gt -> SBUF block tile)
        u -= spk          (tensor_tensor subtract; THR == 1.0)
Bias is folded away via u = mem - b/(1-decay): per-channel spike
threshold + init, zero per-step bias work.

Host-side prep: slice/transpose/pack x and W per core, final output is
a cheap transpose + concat. No collectives; pure SPMD.
"""

import math

import numpy as np

import concourse.bacc as bacc
import concourse.mybir as mybir
import concourse.tile as tile
from concourse import bass_utils

# Problem constants (hardcoded per contract)
T, B, I, O = 64, 128, 2048, 2048
N_CORES = 8
BG, OG = 4, 2              # batch groups x out groups
BL = B // BG               # 32 batches per core
OL = O // OG               # 1024 out channels per core
NOT = OL // 128            # 8 o-tiles
KT = I // 128              # 16 k-tiles
TBL = T * BL               # 2048 columns per core (t-major)
BLK = 256                  # columns per block = 8 timesteps
N_BLK = TBL // BLK         # 8 blocks
TPB = BLK // BL            # 8 timesteps per block
TAU, THR = 2.0, 1.0
DECAY = math.exp(-1.0 / TAU)

F32 = mybir.dt.float32
F32R = mybir.dt.float32r
ALU = mybir.AluOpType

_cache = {}


def _build_nc():
    nc = bacc.Bacc(trn_type="TRN2", target_bir_lowering=False)

    # DRAM I/O (all host-prepacked to the exact SBUF layouts)
    xp_d = nc.dram_tensor("xp", [128, KT, TBL], F32R, kind="ExternalInput")
    wp_d = nc.dram_tensor("wp", [128, KT, NOT, 128], F32R, kind="ExternalInput")
    thr_d = nc.dram_tensor("thr", [128, NOT, BL], F32, kind="ExternalInput")
    u0_d = nc.dram_tensor("u0", [128, NOT, BL], F32, kind="ExternalInput")
    out_d = nc.dram_tensor("out", [128, T, NOT, BL], F32, kind="ExternalOutput")

    with tile.TileContext(nc) as tc:
        with (
            tc.tile_pool(name="wpool", bufs=1) as wpool,
            tc.tile_pool(name="xpool", bufs=2) as xpool,
            tc.tile_pool(name="state", bufs=1) as state,
            tc.tile_pool(name="spkpool", bufs=2) as spkpool,
            tc.tile_pool(name="psum", bufs=2, space="PSUM") as psum_pool,
        ):
            # Persistent state tiles first on the sync queue (small), then
            # x streams behind them. W streams per-k on the gpsimd queue.
            u = state.tile([128, NOT, BL], F32)
            thr_t = state.tile([128, NOT, BL], F32)
            nc.sync.dma_start(u[:], u0_d[:])
            nc.sync.dma_start(thr_t[:], thr_d[:])

            w_all = wpool.tile([128, KT, NOT, 128], F32R)
            for k in range(KT):
                nc.gpsimd.dma_start(w_all[:, k], wp_d[:, k])

            for bi in range(N_BLK):
                cs = bi * BLK
                xt = xpool.tile([128, KT, BLK], F32R, tag="xt")
                nc.sync.dma_start(xt[:], xp_d[:, :, cs:cs + BLK])

                # cur for this block: [of, ot, (t, b)] in PSUM (4 banks)
                # ot outer, k inner: even/odd ot pairs share a PSUM bank,
                # and only one accumulation group per bank may be open.
                ps = psum_pool.tile([128, NOT, BLK], F32, tag="ps")
                for ot in range(NOT):
                    for k in range(KT):
                        nc.tensor.matmul(
                            ps[:, ot],
                            w_all[:, k, ot],
                            xt[:, k],
                            start=(k == 0),
                            stop=(k == KT - 1),
                        )

                # 8 LIF steps consuming this block's PSUM
                spk = spkpool.tile([128, TPB, NOT, BL], F32, tag="spk")
                for tl in range(TPB):
                    cur = ps[:, :, tl * BL:(tl + 1) * BL]  # [128, NOT, BL]
                    nc.vector.scalar_tensor_tensor(
                        u[:], u[:], DECAY, cur, op0=ALU.mult, op1=ALU.add)
                    nc.vector.tensor_tensor(spk[:, tl], u[:], thr_t[:], op=ALU.is_gt)
                    nc.vector.tensor_tensor(u[:], u[:], spk[:, tl], op=ALU.subtract)
                nc.scalar.dma_start(out_d[:, bi * TPB:(bi + 1) * TPB], spk[:])

    nc.compile()
    return nc


def _get_nc():
    if "nc" not in _cache:
        _cache["nc"] = _build_nc()
    return _cache["nc"]


def kernel(x_seq: np.ndarray, W: np.ndarray, b: np.ndarray) -> np.ndarray:
    nc = _get_nc()

    x_seq = np.ascontiguousarray(x_seq, dtype=np.float32)
    W = np.asarray(W, dtype=np.float32)
    b = np.asarray(b, dtype=np.float32)

    # x pack per b-group: [128(p), KT, TBL] with col = t*BL + b_local
    xps = []
    for bg in range(BG):
        xs = x_seq[:, bg * BL:(bg + 1) * BL, :].reshape(TBL, I)
        xps.append(np.ascontiguousarray(
            xs.reshape(TBL, KT, 128).transpose(2, 1, 0)))

    # W pack + folded bias tiles per o-group
    wps, thrs, u0s = [], [], []
    for og in range(OG):
        w_c = W[og * OL:(og + 1) * OL, :]                   # [OL, I]
        wps.append(np.ascontiguousarray(
            w_c.reshape(NOT, 128, KT, 128).transpose(3, 2, 0, 1)))
        b_c = b[og * OL:(og + 1) * OL]
        shift = b_c / (1.0 - DECAY)
        thr2 = (THR - shift).reshape(NOT, 128).T            # [128(of), NOT]
        u02 = (-shift).reshape(NOT, 128).T
        thrs.append(np.ascontiguousarray(
            np.broadcast_to(thr2[:, :, None], (128, NOT, BL)), dtype=np.float32))
        u0s.append(np.ascontiguousarray(
            np.broadcast_to(u02[:, :, None], (128, NOT, BL)), dtype=np.float32))

    in_maps = []
    for c in range(N_CORES):
        og, bg = c // BG, c % BG
        in_maps.append({
            "xp": xps[bg], "wp": wps[og], "thr": thrs[og], "u0": u0s[og],
        })

    res = bass_utils.run_bass_kernel_spmd(nc, in_maps, core_ids=list(range(N_CORES)))
    global LAST_RESULT
    LAST_RESULT = res

    # Assemble: out_c[of, t, ot, b] -> full[t, b, o]
    full = np.empty((T, B, O), dtype=np.float32)
    for c in range(N_CORES):
        og, bg = c // BG, c % BG
        oc = res.results[c]["out"]                          # [128, T, NOT, BL]
        full[:, bg * BL:(bg + 1) * BL, og * OL:(og + 1) * OL] = (
            oc.transpose(1, 3, 2, 0).reshape(T, BL, OL))
    return full


LAST_RESULT = None
